# revision 14
# baseline (speedup 1.0000x reference)
"""AdaConv (per-sample dynamic grouped conv) on 8 TRN2 NeuronCores.

Data parallel: batch 16 -> 8 cores x 2 samples. Per core:
  - predictor convs (tiny matmuls, bf16) produce per-sample dw (512,8,3,3)
    and pw (512,8) kernels + bias (512,), laid out directly as block-diagonal
    128x128 lhsT tiles via small scatter DMAs
  - instance norm (DVE/ACT) + reflect pad into a [128, 66, 66] SBUF tile
  - 3x3 grouped conv = 9 shifted-AP matmuls (f32r) accumulated in PSUM,
    pointwise grouped conv = 1 matmul, Lrelu(+bias) on ACT, +cn on DVE
Channel tiling: 512 ch = 4 tiles x 128 partitions; group g (8ch) is tile-local.
"""

import numpy as np

N_FULL = 16
N_CORES = 8
NPC = 2            # samples per core
TS = 4             # channel tiles of 128
CH = 512
HW = 64
SP = HW * HW       # 4096
PADW = HW + 2      # 66
EPS = 1e-5
NCHUNK = 8         # spatial chunks of 512 px (8 rows)
CK = SP // NCHUNK  # 512


def _host_prep(style_encoding, content_in, dw_w, dw_b, pw_kn_w, pw_kn_b,
               pw_bias_w, pw_bias_b):
    """Layout-only transforms. Returns list of 8 per-core input dicts."""
    import ml_dtypes
    bf16 = ml_dtypes.bfloat16

    ts_i = np.arange(TS)[:, None, None]
    o_i = np.arange(8)[None, :, None]
    m_i = np.arange(128)[None, None, :]
    OC = 8 * (128 * ts_i + 8 * (m_i // 8) + o_i) + (m_i % 8)  # (4,8,128)

    # dw predictor weights -> block-diag-of-two-64 lhsT tiles (bf16)
    W = dw_w[OC]                                  # (4,8,128m,64ic,2,2)
    Wk = np.transpose(W, (0, 1, 4, 5, 3, 2)).reshape(TS, 8, 4, 64, 128)
    dwp = np.zeros((TS, 8, 4, 128, 128), np.float32)
    dwp[:, :, :, 0:64, 0:64] = Wk[:, :, :, :, 0:64]
    dwp[:, :, :, 64:128, 64:128] = Wk[:, :, :, :, 64:128]
    dwp = dwp.reshape(32, 4, 128, 128).astype(bf16)

    # pw predictor weights (scaled by 1/16 to fold the style spatial mean)
    Wp = np.transpose(pw_kn_w[OC, :, 0, 0] / 16.0, (0, 1, 3, 2))  # (4,8,64,128)
    pwp = np.zeros((TS, 8, 128, 128), np.float32)
    pwp[:, :, 0:64, 0:64] = Wp[:, :, :, 0:64]
    pwp[:, :, 64:128, 64:128] = Wp[:, :, :, 64:128]
    pwp = pwp.reshape(32, 128, 128).astype(bf16)

    pwbT = np.ascontiguousarray(
        (pw_bias_w[:, :, 0, 0].T / 16.0).reshape(TS, 128, CH)
        .transpose(1, 0, 2)).astype(bf16)                        # (128,4,512)

    p_i = np.arange(128)[:, None, None]
    OC2 = 8 * (128 * np.arange(TS)[None, :, None] + 8 * (p_i // 8)
               + np.arange(8)[None, None, :]) + (p_i % 8)        # (128,4,8)
    dwb = dw_b[OC2].reshape(128, 32).astype(np.float32)
    pwkb = pw_kn_b[OC2].reshape(128, 32).astype(np.float32)
    pbb = np.ascontiguousarray(pw_bias_b.reshape(TS, 128).T)     # (128,4)

    in_maps = []
    for c in range(N_CORES):
        sl = slice(c * NPC, (c + 1) * NPC)
        style_core = np.asarray(style_encoding[sl])              # (2,512,4,4)
        sd = style_core.reshape(NPC, TS, 128, 4, 4).transpose(2, 1, 0, 3, 4)
        content_core = np.ascontiguousarray(
            np.asarray(content_in[sl]).reshape(NPC, TS, 128, SP))
        in_maps.append({
            "style": np.ascontiguousarray(sd).astype(bf16),
            "content": content_core.astype(np.float32),
            "dwp": dwp, "pwp": pwp, "pwbT": pwbT,
            "dwb": dwb, "pwkb": pwkb, "pbb": pbb,
            "zeros": np.zeros((128, 10240), np.float32),
        })
    return in_maps


def _emulate_core(m):
    """Numpy mimic of the device graph (f32) for one core's in_map."""
    style = np.asarray(m["style"], np.float32).transpose(1, 0, 2, 3, 4)  # (4,128,2,4,4)
    content = m["content"]                         # (2,4,128,4096)
    dwp = np.asarray(m["dwp"], np.float32).reshape(TS, 8, 4, 128, 128)
    pwp = np.asarray(m["pwp"], np.float32).reshape(TS, 8, 128, 128)
    pwbT = np.asarray(m["pwbT"], np.float32)
    dwb, pwkb, pbb = m["dwb"], m["pwkb"], m["pbb"]

    sd = style.sum(axis=(3, 4))                    # (4,128,2)  (sum, not mean)

    lhsT = np.zeros((128, TS, NPC, 9, 16, 8), np.float32)
    pwlT = np.zeros((128, TS, NPC, 16, 8), np.float32)
    bias_sb = np.zeros((128, TS, NPC), np.float32)

    for ts in range(TS):
        for o in range(8):
            mt = ts * 8 + o
            psA = np.zeros((128, NPC, 3, 3), np.float32)
            for koff in range(4):
                ky, kx = divmod(koff, 2)
                rhs = style[ts, :, :, ky:ky + 3, kx:kx + 3]     # (128,2,3,3)
                psA += np.einsum('rm,rnab->mnab', dwp[ts, o, koff], rhs)
            S = np.maximum(psA + dwb[:, mt][:, None, None, None], 0.0)
            psB = np.einsum('rm,rn->mn', pwp[ts, o], sd[ts])    # (128,2)
            Spw = np.maximum(psB + pwkb[:, mt][:, None], 0.0)
            for G in range(16):
                r = slice(8 * G, 8 * G + 8)
                # S[p,n,dy,dx] -> lhsT[p, ts, n, k, G, o]
                lhsT[r, ts, :, :, G, o] = S[r].reshape(8, NPC, 9)
                pwlT[r, ts, :, G, o] = Spw[r]
    for ts in range(TS):
        psC = np.zeros((128, NPC), np.float32)
        for kt in range(TS):
            psC += np.einsum('rm,rn->mn',
                             pwbT[:, kt, 128 * ts:128 * (ts + 1)], sd[kt])
        bias_sb[:, ts, :] = np.maximum(psC + pbb[:, ts][:, None], 0.0)

    out = np.zeros((NPC, TS, 128, SP), np.float32)
    for n in range(NPC):
        for ts in range(TS):
            x = content[n, ts]                                  # (128,4096)
            s = x.sum(axis=1)
            sq = (x * x).sum(axis=1)
            mean = s / SP
            var = (sq - mean * s) / (SP - 1)
            rstd = 1.0 / np.sqrt(var + EPS)
            cn = (x - mean[:, None]) * rstd[:, None]
            cn2 = cn.reshape(128, HW, HW)
            pad = np.zeros((128, PADW, PADW), np.float32)
            pad[:, 1:65, 1:65] = cn2
            pad[:, 0, 1:65] = pad[:, 2, 1:65]
            pad[:, 65, 1:65] = pad[:, 63, 1:65]
            pad[:, :, 0] = pad[:, :, 2]
            pad[:, :, 65] = pad[:, :, 63]
            for c in range(NCHUNK):
                psD = np.zeros((128, 8, 64), np.float32)
                for k in range(9):
                    dy, dx = divmod(k, 3)
                    W128 = np.zeros((128, 128), np.float32)
                    W128 = lhsT[:, ts, n, k, :, :].reshape(128, 128)
                    rhs = pad[:, c * 8 + dy:c * 8 + dy + 8, dx:dx + 64]
                    psD += np.einsum('rm,rab->mab', W128, rhs)
                d = psD.reshape(128, CK)
                psE = np.einsum('rm,rq->mq', pwlT[:, ts, n].reshape(128, 128), d)
                t = psE + bias_sb[:, ts, n][:, None]
                t = np.where(t >= 0, t, 0.01 * t)
                out[n, ts, :, c * CK:(c + 1) * CK] = \
                    t + cn[:, c * CK:(c + 1) * CK]
    return out


def _build_nc():
    import concourse.bass as bass
    import concourse.mybir as mybir
    from concourse import bacc
    from concourse.tile import TileContext

    f32, bf, f32r = mybir.dt.float32, mybir.dt.bfloat16, mybir.dt.float32r
    AF = mybir.ActivationFunctionType
    ALU = mybir.AluOpType
    AX = mybir.AxisListType

    nc = bacc.Bacc()
    style_d = nc.declare_dram_parameter("style", [128, TS, NPC, 4, 4], bf, False)
    content_d = nc.declare_dram_parameter("content", [NPC, TS, 128, SP], f32r, False)
    dwp_d = nc.declare_dram_parameter("dwp", [32, 4, 128, 128], bf, False)
    pwp_d = nc.declare_dram_parameter("pwp", [32, 128, 128], bf, False)
    pwbT_d = nc.declare_dram_parameter("pwbT", [128, TS, CH], bf, False)
    dwb_d = nc.declare_dram_parameter("dwb", [128, 32], f32, False)
    pwkb_d = nc.declare_dram_parameter("pwkb", [128, 32], f32, False)
    pbb_d = nc.declare_dram_parameter("pbb", [128, TS], f32, False)
    zeros_d = nc.declare_dram_parameter("zeros", [128, 10240], f32r, False)
    out_d = nc.declare_dram_parameter("out", [NPC, TS, 128, SP], f32, True)

    with TileContext(nc) as tc:
        with (
            tc.tile_pool(name="persist", bufs=1) as pp,
            tc.tile_pool(name="wstream", bufs=3) as wp,
            tc.tile_pool(name="pads", bufs=2) as padp,
            tc.tile_pool(name="work", bufs=3) as wkp,
            tc.tile_pool(name="stats", bufs=8) as stp,
            tc.tile_pool(name="psA", bufs=2, space="PSUM") as psa,
            tc.tile_pool(name="psD", bufs=2, space="PSUM") as psd,
            tc.tile_pool(name="psE", bufs=2, space="PSUM") as pse,
        ):
            style_sb = pp.tile([128, TS, NPC, 4, 4], bf, tag="style")
            dwb_sb = pp.tile([128, 32], f32, tag="dwb")
            pwkb_sb = pp.tile([128, 32], f32, tag="pwkb")
            pbb_sb = pp.tile([128, TS], f32, tag="pbb")
            pwbT_sb = pp.tile([128, TS, CH], bf, tag="pwbT")
            sd_f = pp.tile([128, TS, NPC], f32, tag="sdf")
            sd_sb = pp.tile([128, TS, NPC], bf, tag="sd")
            lhsT = pp.tile([128, TS, NPC, 9, 16, 8], f32r, tag="lhsT")
            pwlT = pp.tile([128, TS, NPC, 16, 8], f32r, tag="pwlT")
            S_dw = pp.tile([128, TS, NPC, 9, 8], f32r, tag="Sdw")
            S_pw = pp.tile([128, TS, NPC, 8], f32r, tag="Spw")
            bias_sb = pp.tile([128, TS, NPC], f32, tag="bias")
            eps_sb = pp.tile([128, 1], f32, tag="eps")
            nc.vector.memset(eps_sb[:], EPS)

            nc.sync.dma_start(
                out=style_sb[:].rearrange("p a n y x -> p (a n y x)"),
                in_=style_d[:].rearrange("p a n y x -> p (a n y x)"))
            nc.sync.dma_start(out=dwb_sb[:], in_=dwb_d[:])
            nc.sync.dma_start(out=pwkb_sb[:], in_=pwkb_d[:])
            nc.sync.dma_start(out=pbb_sb[:], in_=pbb_d[:])
            nc.sync.dma_start(
                out=pwbT_sb[:].rearrange("p a b -> p (a b)"),
                in_=pwbT_d[:].rearrange("p a b -> p (a b)"))

            nc.sync.dma_start(
                out=lhsT[:].rearrange("p a n k g o -> p (a n k g o)"),
                in_=zeros_d[:, 0:9216])
            nc.sync.dma_start(
                out=pwlT[:].rearrange("p a n g o -> p (a n g o)"),
                in_=zeros_d[:, 9216:10240])

            # style spatial sum -> sd (x 1/16 folded into pw weights)
            for ts in range(TS):
                nc.vector.tensor_reduce(
                    out=sd_f[:, ts, :], in_=style_sb[:, ts, :, :, :],
                    op=ALU.add, axis=AX.XY)
            nc.vector.tensor_copy(
                sd_sb[:].rearrange("p a n -> p (a n)"),
                sd_f[:].rearrange("p a n -> p (a n)"))

            # ---- predictor matmuls ----
            for ts in range(TS):
                for o in range(8):
                    mt = ts * 8 + o
                    dmat = wp.tile([128, 4, 128], bf, tag="dmat")
                    nc.sync.dma_start(out=dmat[:],
                                      in_=dwp_d[mt].transpose([1, 0, 2]))
                    ps = psa.tile([128, NPC, 3, 3], f32, tag="psA")
                    for koff in range(4):
                        ky, kx = divmod(koff, 2)
                        nc.tensor.matmul(
                            ps[:], dmat[:, koff, :],
                            style_sb[:, ts, :, ky:ky + 3, kx:kx + 3],
                            start=(koff == 0), stop=(koff == 3))
                    nc.scalar.activation(
                        S_dw[:, ts, :, :, o], ps[:], AF.Relu,
                        bias=dwb_sb[:, mt:mt + 1])

                    pmat = wp.tile([128, 128], bf, tag="pmat")
                    nc.sync.dma_start(out=pmat[:], in_=pwp_d[mt])
                    ps2 = psa.tile([128, NPC], f32, tag="psA")
                    nc.tensor.matmul(ps2[:], pmat[:], sd_sb[:, ts, :],
                                     start=True, stop=True)
                    nc.scalar.activation(
                        S_pw[:, ts, :, o], ps2[:], AF.Relu,
                        bias=pwkb_sb[:, mt:mt + 1])

            for ts in range(TS):
                ps3 = psa.tile([128, NPC], f32, tag="psA")
                for kt in range(TS):
                    nc.tensor.matmul(
                        ps3[:], pwbT_sb[:, kt, 128 * ts:128 * (ts + 1)],
                        sd_sb[:, kt, :], start=(kt == 0), stop=(kt == 3))
                nc.scalar.activation(bias_sb[:, ts, :], ps3[:], AF.Relu,
                                     bias=pbb_sb[:, ts:ts + 1])

            # ---- scatter into block-diagonal lhsT tiles ----
            for ts in range(TS):
                for G in range(16):
                    r = slice(8 * G, 8 * G + 8)
                    nc.sync.dma_start(out=lhsT[r, ts, :, :, G, :],
                                      in_=S_dw[r, ts, :, :, :])
                    nc.sync.dma_start(out=pwlT[r, ts, :, G, :],
                                      in_=S_pw[r, ts, :, :])

            # ---- main per (n, ts) ----
            for n in range(NPC):
                for ts in range(TS):
                    pad = padp.tile([128, PADW, PADW], f32r, tag="pad")
                    interior = pad[:, 1:65, 1:65]
                    nc.sync.dma_start(out=interior, in_=content_d[n, ts])

                    s_t = stp.tile([128, 1], f32, tag="sum")
                    sq_t = stp.tile([128, 1], f32, tag="sumsq")
                    nc.vector.tensor_reduce(out=s_t[:], in_=interior,
                                            op=ALU.add, axis=AX.XY)
                    sqscr = wkp.tile([128, CK], f32, tag="sqscr")
                    sqp = stp.tile([128, NCHUNK], f32, tag="sqp")
                    for c in range(NCHUNK):
                        chunk = pad[:, 1 + c * 8:1 + c * 8 + 8, 1:65]
                        nc.vector.scalar_tensor_tensor(
                            out=sqscr[:].rearrange("p (a b) -> p a b", a=8),
                            in0=chunk, scalar=1.0, in1=chunk,
                            op0=ALU.mult, op1=ALU.mult,
                            accum_out=sqp[:, c:c + 1])
                    nc.vector.tensor_reduce(out=sq_t[:], in_=sqp[:],
                                            op=ALU.add, axis=AX.X)
                    mean = stp.tile([128, 1], f32, tag="mean")
                    nc.vector.tensor_scalar(out=mean[:], in0=s_t[:],
                                            scalar1=1.0 / SP, scalar2=None,
                                            op0=ALU.mult)
                    msq = stp.tile([128, 1], f32, tag="msq")
                    nc.vector.scalar_tensor_tensor(
                        out=msq[:], in0=mean[:], scalar=1.0, in1=s_t[:],
                        op0=ALU.mult, op1=ALU.mult)
                    var_ = stp.tile([128, 1], f32, tag="var")
                    nc.vector.scalar_tensor_tensor(
                        out=var_[:], in0=sq_t[:], scalar=1.0, in1=msq[:],
                        op0=ALU.mult, op1=ALU.subtract)
                    std = stp.tile([128, 1], f32, tag="std")
                    nc.scalar.activation(std[:], var_[:], AF.Sqrt,
                                         bias=eps_sb[:, 0:1],
                                         scale=1.0 / (SP - 1))
                    rstd = stp.tile([128, 1], f32, tag="rstd")
                    nc.vector.reciprocal(rstd[:], std[:])
                    nshift = stp.tile([128, 1], f32, tag="nshift")
                    nc.vector.scalar_tensor_tensor(
                        out=nshift[:], in0=mean[:], scalar=-1.0, in1=rstd[:],
                        op0=ALU.mult, op1=ALU.mult)
                    # normalize in place: x*rstd + nshift (DVE)
                    nc.vector.tensor_scalar(
                        out=interior, in0=interior,
                        scalar1=rstd[:, 0:1], scalar2=nshift[:, 0:1],
                        op0=ALU.mult, op1=ALU.add)
                    # reflect borders
                    nc.vector.tensor_copy(pad[:, 0, 1:65], pad[:, 2, 1:65])
                    nc.vector.tensor_copy(pad[:, 65, 1:65], pad[:, 63, 1:65])
                    nc.vector.tensor_copy(pad[:, :, 0], pad[:, :, 2])
                    nc.vector.tensor_copy(pad[:, :, 65], pad[:, :, 63])

                    wT = lhsT[:, ts, n].rearrange("p a b c -> p a (b c)")
                    pwT = pwlT[:, ts, n].rearrange("p a b -> p (a b)")
                    for c in range(NCHUNK):
                        ps4 = psd.tile([128, 8, 64], f32, tag="psD")
                        for k in range(9):
                            dy, dx = divmod(k, 3)
                            nc.tensor.matmul(
                                ps4[:], wT[:, k, :],
                                pad[:, c * 8 + dy:c * 8 + dy + 8,
                                    dx:dx + 64],
                                start=(k == 0), stop=(k == 8))
                        dsb = wkp.tile([128, CK], f32r, tag="dsb")
                        nc.vector.tensor_copy(
                            dsb[:].rearrange("p (a b) -> p a b", a=8), ps4[:])
                        ps5 = pse.tile([128, CK], f32, tag="psE")
                        nc.tensor.matmul(ps5[:], pwT, dsb[:],
                                         start=True, stop=True)
                        ot = wkp.tile([128, CK], f32, tag="ot")
                        u = wkp.tile([128, CK], f32, tag="u")
                        nc.vector.tensor_scalar(
                            out=u[:], in0=ps5[:],
                            scalar1=bias_sb[:, ts, n:n + 1], scalar2=0.01,
                            op0=ALU.add, op1=ALU.mult)
                        nc.vector.scalar_tensor_tensor(
                            out=ot[:], in0=ps5[:],
                            scalar=bias_sb[:, ts, n:n + 1], op0=ALU.add,
                            in1=u[:], op1=ALU.max)
                        nc.vector.scalar_tensor_tensor(
                            out=ot[:].rearrange("p (a b) -> p a b", a=8),
                            in0=ot[:].rearrange("p (a b) -> p a b", a=8),
                            scalar=0.0, op0=ALU.add,
                            in1=pad[:, 1 + c * 8:1 + c * 8 + 8, 1:65],
                            op1=ALU.add)
                        nc.sync.dma_start(out=out_d[n, ts, :, c * CK:(c + 1) * CK],
                                          in_=ot[:])
    nc.compile()
    return nc


_NC_CACHE = None


def kernel(**inputs):
    global _NC_CACHE
    in_maps = _host_prep(**inputs)
    if _NC_CACHE is None:
        _NC_CACHE = _build_nc()
    nc = _NC_CACHE
    from concourse.bass_utils import run_bass_kernel_spmd
    res = run_bass_kernel_spmd(nc, in_maps, core_ids=list(range(N_CORES)))
    outs = []
    for c in range(N_CORES):
        o = res.results[c]["out"].reshape(NPC, TS, 128, SP)
        outs.append(o.reshape(NPC, CH, HW, HW))
    return np.concatenate(outs, axis=0).astype(np.float32)


# revision 17
# speedup vs baseline: 1.2974x; 1.2974x over previous
"""AdaConv (per-sample dynamic grouped conv) on 8 TRN2 NeuronCores.

Data parallel: batch 16 -> 8 cores x 2 samples. Per core:
  - predictor convs (tiny matmuls, bf16) produce per-sample dw (512,8,3,3)
    and pw (512,8) kernels + bias (512,), laid out directly as block-diagonal
    128x128 lhsT tiles via small scatter DMAs
  - instance norm (DVE/ACT) + reflect pad into a [128, 66, 66] SBUF tile
  - 3x3 grouped conv = 9 shifted-AP matmuls (f32r) accumulated in PSUM,
    pointwise grouped conv = 1 matmul, Lrelu(+bias) on ACT, +cn on DVE
Channel tiling: 512 ch = 4 tiles x 128 partitions; group g (8ch) is tile-local.
"""

import numpy as np

N_FULL = 16
N_CORES = 8
NPC = 2            # samples per core
TS = 4             # channel tiles of 128
CH = 512
HW = 64
SP = HW * HW       # 4096
PADW = HW + 2      # 66
EPS = 1e-5
NCHUNK = 8         # spatial chunks of 512 px (8 rows)
CK = SP // NCHUNK  # 512


def _host_prep(style_encoding, content_in, dw_w, dw_b, pw_kn_w, pw_kn_b,
               pw_bias_w, pw_bias_b):
    """Layout-only transforms. Returns list of 8 per-core input dicts."""
    import ml_dtypes
    bf16 = ml_dtypes.bfloat16

    ts_i = np.arange(TS)[:, None, None]
    o_i = np.arange(8)[None, :, None]
    m_i = np.arange(128)[None, None, :]
    OC = 8 * (128 * ts_i + 8 * (m_i // 8) + o_i) + (m_i % 8)  # (4,8,128)

    # dw predictor weights -> block-diag-of-two-64 lhsT tiles (bf16)
    W = dw_w[OC]                                  # (4,8,128m,64ic,2,2)
    Wk = np.transpose(W, (0, 1, 4, 5, 3, 2)).reshape(TS, 8, 4, 64, 128)
    dwp = np.zeros((TS, 8, 4, 128, 128), np.float32)
    dwp[:, :, :, 0:64, 0:64] = Wk[:, :, :, :, 0:64]
    dwp[:, :, :, 64:128, 64:128] = Wk[:, :, :, :, 64:128]
    dwp = dwp.reshape(32, 4, 128, 128).astype(bf16)

    # pw predictor weights (scaled by 1/16 to fold the style spatial mean)
    Wp = np.transpose(pw_kn_w[OC, :, 0, 0] / 16.0, (0, 1, 3, 2))  # (4,8,64,128)
    pwp = np.zeros((TS, 8, 128, 128), np.float32)
    pwp[:, :, 0:64, 0:64] = Wp[:, :, :, 0:64]
    pwp[:, :, 64:128, 64:128] = Wp[:, :, :, 64:128]
    pwp = pwp.reshape(32, 128, 128).astype(bf16)

    pwbT = np.ascontiguousarray(
        (pw_bias_w[:, :, 0, 0].T / 16.0).reshape(TS, 128, CH)
        .transpose(1, 0, 2)).astype(bf16)                        # (128,4,512)

    p_i = np.arange(128)[:, None, None]
    OC2 = 8 * (128 * np.arange(TS)[None, :, None] + 8 * (p_i // 8)
               + np.arange(8)[None, None, :]) + (p_i % 8)        # (128,4,8)
    dwb = dw_b[OC2].reshape(128, 32).astype(np.float32)
    pwkb = pw_kn_b[OC2].reshape(128, 32).astype(np.float32)
    pbb = np.ascontiguousarray(pw_bias_b.reshape(TS, 128).T)     # (128,4)

    in_maps = []
    for c in range(N_CORES):
        sl = slice(c * NPC, (c + 1) * NPC)
        style_core = np.asarray(style_encoding[sl])              # (2,512,4,4)
        sd = style_core.reshape(NPC, TS, 128, 4, 4).transpose(2, 1, 0, 3, 4)
        content_core = np.ascontiguousarray(
            np.asarray(content_in[sl]).reshape(NPC, TS, 128, SP))
        in_maps.append({
            "style": np.ascontiguousarray(sd).astype(bf16),
            "content": content_core.astype(np.float32),
            "dwp": dwp, "pwp": pwp, "pwbT": pwbT,
            "dwb": dwb, "pwkb": pwkb, "pbb": pbb,
            "zeros": np.zeros((128, 9216), np.float32),
        })
    return in_maps


def _emulate_core(m):
    """Numpy mimic of the device graph (f32) for one core's in_map."""
    style = np.asarray(m["style"], np.float32).transpose(1, 0, 2, 3, 4)  # (4,128,2,4,4)
    content = m["content"]                         # (2,4,128,4096)
    dwp = np.asarray(m["dwp"], np.float32).reshape(TS, 8, 4, 128, 128)
    pwp = np.asarray(m["pwp"], np.float32).reshape(TS, 8, 128, 128)
    pwbT = np.asarray(m["pwbT"], np.float32)
    dwb, pwkb, pbb = m["dwb"], m["pwkb"], m["pbb"]

    sd = style.sum(axis=(3, 4))                    # (4,128,2)  (sum, not mean)

    lhsT = np.zeros((128, TS, NPC, 9, 16, 8), np.float32)
    pwlT = np.zeros((128, TS, NPC, 16, 8), np.float32)
    bias_sb = np.zeros((128, TS, NPC), np.float32)

    for ts in range(TS):
        for o in range(8):
            mt = ts * 8 + o
            psA = np.zeros((128, NPC, 3, 3), np.float32)
            for koff in range(4):
                ky, kx = divmod(koff, 2)
                rhs = style[ts, :, :, ky:ky + 3, kx:kx + 3]     # (128,2,3,3)
                psA += np.einsum('rm,rnab->mnab', dwp[ts, o, koff], rhs)
            S = np.maximum(psA + dwb[:, mt][:, None, None, None], 0.0)
            psB = np.einsum('rm,rn->mn', pwp[ts, o], sd[ts])    # (128,2)
            Spw = np.maximum(psB + pwkb[:, mt][:, None], 0.0)
            for G in range(16):
                r = slice(8 * G, 8 * G + 8)
                # S[p,n,dy,dx] -> lhsT[p, ts, n, k, G, o]
                lhsT[r, ts, :, :, G, o] = S[r].reshape(8, NPC, 9)
                pwlT[r, ts, :, G, o] = Spw[r]
    for ts in range(TS):
        psC = np.zeros((128, NPC), np.float32)
        for kt in range(TS):
            psC += np.einsum('rm,rn->mn',
                             pwbT[:, kt, 128 * ts:128 * (ts + 1)], sd[kt])
        bias_sb[:, ts, :] = np.maximum(psC + pbb[:, ts][:, None], 0.0)

    out = np.zeros((NPC, TS, 128, SP), np.float32)
    for n in range(NPC):
        for ts in range(TS):
            x = content[n, ts]                                  # (128,4096)
            s = x.sum(axis=1)
            sq = (x * x).sum(axis=1)
            mean = s / SP
            var = (sq - mean * s) / (SP - 1)
            rstd = 1.0 / np.sqrt(var + EPS)
            cn = (x - mean[:, None]) * rstd[:, None]
            cn2 = cn.reshape(128, HW, HW)
            pad = np.zeros((128, PADW, PADW), np.float32)
            pad[:, 1:65, 1:65] = cn2
            pad[:, 0, 1:65] = pad[:, 2, 1:65]
            pad[:, 65, 1:65] = pad[:, 63, 1:65]
            pad[:, :, 0] = pad[:, :, 2]
            pad[:, :, 65] = pad[:, :, 63]
            for c in range(NCHUNK):
                psD = np.zeros((128, 8, 64), np.float32)
                for k in range(9):
                    dy, dx = divmod(k, 3)
                    W128 = np.zeros((128, 128), np.float32)
                    W128 = lhsT[:, ts, n, k, :, :].reshape(128, 128)
                    rhs = pad[:, c * 8 + dy:c * 8 + dy + 8, dx:dx + 64]
                    psD += np.einsum('rm,rab->mab', W128, rhs)
                d = psD.reshape(128, CK)
                psE = np.einsum('rm,rq->mq', pwlT[:, ts, n].reshape(128, 128), d)
                t = psE + bias_sb[:, ts, n][:, None]
                t = np.where(t >= 0, t, 0.01 * t)
                out[n, ts, :, c * CK:(c + 1) * CK] = \
                    t + cn[:, c * CK:(c + 1) * CK]
    return out


def _build_nc():
    import concourse.bass as bass
    import concourse.mybir as mybir
    from concourse import bacc
    from concourse.tile import TileContext

    f32, bf, f32r = mybir.dt.float32, mybir.dt.bfloat16, mybir.dt.float32r
    AF = mybir.ActivationFunctionType
    ALU = mybir.AluOpType
    AX = mybir.AxisListType

    nc = bacc.Bacc()
    style_d = nc.declare_dram_parameter("style", [128, TS, NPC, 4, 4], bf, False)
    content_d = nc.declare_dram_parameter("content", [NPC, TS, 128, SP], f32r, False)
    dwp_d = nc.declare_dram_parameter("dwp", [32, 4, 128, 128], bf, False)
    pwp_d = nc.declare_dram_parameter("pwp", [32, 128, 128], bf, False)
    pwbT_d = nc.declare_dram_parameter("pwbT", [128, TS, CH], bf, False)
    dwb_d = nc.declare_dram_parameter("dwb", [128, 32], f32, False)
    pwkb_d = nc.declare_dram_parameter("pwkb", [128, 32], f32, False)
    pbb_d = nc.declare_dram_parameter("pbb", [128, TS], f32, False)
    zeros_d = nc.declare_dram_parameter("zeros", [128, 9216], f32r, False)
    out_d = nc.declare_dram_parameter("out", [NPC, TS, 128, SP], f32, True)

    with TileContext(nc) as tc:
        with (
            tc.tile_pool(name="persist", bufs=1) as pp,
            tc.tile_pool(name="wstream", bufs=3) as wp,
            tc.tile_pool(name="pads", bufs=2) as padp,
            tc.tile_pool(name="work", bufs=3) as wkp,
            tc.tile_pool(name="stats", bufs=8) as stp,
            tc.tile_pool(name="psA", bufs=2, space="PSUM") as psa,
            tc.tile_pool(name="psD", bufs=2, space="PSUM") as psd,
            tc.tile_pool(name="psE", bufs=2, space="PSUM") as pse,
        ):
            style_sb = pp.tile([128, TS, NPC, 4, 4], bf, tag="style")
            dwb_sb = pp.tile([128, 32], f32, tag="dwb")
            pwkb_sb = pp.tile([128, 32], f32, tag="pwkb")
            pbb_sb = pp.tile([128, TS], f32, tag="pbb")
            pwbT_sb = pp.tile([128, TS, CH], bf, tag="pwbT")
            sd_f = pp.tile([128, TS, NPC], f32, tag="sdf")
            sd_sb = pp.tile([128, TS, NPC], bf, tag="sd")
            lhsT = pp.tile([128, TS, NPC, 9, 16, 8], f32r, tag="lhsT")
            pwlT = pp.tile([128, TS, NPC, 16, 8], bf, tag="pwlT")
            S_dw = pp.tile([128, TS, NPC, 9, 8], f32r, tag="Sdw")
            S_pw = pp.tile([128, TS, NPC, 8], bf, tag="Spw")
            bias_sb = pp.tile([128, TS, NPC], f32, tag="bias")
            negb_sb = pp.tile([128, TS, NPC], f32, tag="negb")
            eps_sb = pp.tile([128, 1], f32, tag="eps")
            nc.vector.memset(eps_sb[:], EPS)

            nc.sync.dma_start(
                out=style_sb[:].rearrange("p a n y x -> p (a n y x)"),
                in_=style_d[:].rearrange("p a n y x -> p (a n y x)"))
            nc.sync.dma_start(out=dwb_sb[:], in_=dwb_d[:])
            nc.sync.dma_start(out=pwkb_sb[:], in_=pwkb_d[:])
            nc.sync.dma_start(out=pbb_sb[:], in_=pbb_d[:])
            nc.sync.dma_start(
                out=pwbT_sb[:].rearrange("p a b -> p (a b)"),
                in_=pwbT_d[:].rearrange("p a b -> p (a b)"))

            nc.sync.dma_start(
                out=lhsT[:].rearrange("p a n k g o -> p (a n k g o)"),
                in_=zeros_d[:, 0:9216])
            nc.vector.memset(pwlT[:].rearrange("p a n g o -> p (a n g o)"), 0.0)

            # style spatial sum -> sd (x 1/16 folded into pw weights)
            for ts in range(TS):
                nc.vector.tensor_reduce(
                    out=sd_f[:, ts, :], in_=style_sb[:, ts, :, :, :],
                    op=ALU.add, axis=AX.XY)
            nc.vector.tensor_copy(
                sd_sb[:].rearrange("p a n -> p (a n)"),
                sd_f[:].rearrange("p a n -> p (a n)"))

            # ---- predictor matmuls ----
            for ts in range(TS):
                for o in range(8):
                    mt = ts * 8 + o
                    dmat = wp.tile([128, 4, 128], bf, tag="dmat")
                    nc.sync.dma_start(out=dmat[:],
                                      in_=dwp_d[mt].transpose([1, 0, 2]))
                    ps = psa.tile([128, NPC, 3, 3], f32, tag="psA")
                    for koff in range(4):
                        ky, kx = divmod(koff, 2)
                        nc.tensor.matmul(
                            ps[:], dmat[:, koff, :],
                            style_sb[:, ts, :, ky:ky + 3, kx:kx + 3],
                            start=(koff == 0), stop=(koff == 3))
                    nc.scalar.activation(
                        S_dw[:, ts, :, :, o], ps[:], AF.Relu,
                        bias=dwb_sb[:, mt:mt + 1])

                    pmat = wp.tile([128, 128], bf, tag="pmat")
                    nc.sync.dma_start(out=pmat[:], in_=pwp_d[mt])
                    ps2 = psa.tile([128, NPC], f32, tag="psA")
                    nc.tensor.matmul(ps2[:], pmat[:], sd_sb[:, ts, :],
                                     start=True, stop=True)
                    nc.scalar.activation(
                        S_pw[:, ts, :, o], ps2[:], AF.Relu,
                        bias=pwkb_sb[:, mt:mt + 1])

            for ts in range(TS):
                ps3 = psa.tile([128, NPC], f32, tag="psA")
                for kt in range(TS):
                    nc.tensor.matmul(
                        ps3[:], pwbT_sb[:, kt, 128 * ts:128 * (ts + 1)],
                        sd_sb[:, kt, :], start=(kt == 0), stop=(kt == 3))
                nc.scalar.activation(bias_sb[:, ts, :], ps3[:], AF.Relu,
                                     bias=pbb_sb[:, ts:ts + 1])
                nc.vector.tensor_scalar(out=negb_sb[:, ts, :],
                                        in0=bias_sb[:, ts, :], scalar1=-1.0,
                                        scalar2=None, op0=ALU.mult)

            # ---- scatter into block-diagonal lhsT tiles ----
            for ts in range(TS):
                for G in range(16):
                    r = slice(8 * G, 8 * G + 8)
                    nc.sync.dma_start(out=lhsT[r, ts, :, :, G, :],
                                      in_=S_dw[r, ts, :, :, :])
                    nc.sync.dma_start(out=pwlT[r, ts, :, G, :],
                                      in_=S_pw[r, ts, :, :])

            # ---- main per (n, ts) ----
            for n in range(NPC):
                for ts in range(TS):
                    pad = padp.tile([128, PADW, PADW], f32r, tag="pad")
                    interior = pad[:, 1:65, 1:65]
                    ctile = padp.tile([128, SP], f32r, tag="ctile")
                    nc.sync.dma_start(out=ctile[:], in_=content_d[n, ts])

                    s_t = stp.tile([128, 1], f32, tag="sum")
                    sq_t = stp.tile([128, 1], f32, tag="sumsq")
                    nc.vector.tensor_reduce(out=s_t[:], in_=ctile[:],
                                            op=ALU.add, axis=AX.X)
                    sqscr = wkp.tile([128, CK], f32, tag="sqscr")
                    sqp = stp.tile([128, NCHUNK], f32, tag="sqp")
                    for c in range(NCHUNK):
                        nc.scalar.activation(
                            sqscr[:], ctile[:, c * CK:(c + 1) * CK],
                            AF.Square, accum_out=sqp[:, c:c + 1])
                    nc.vector.tensor_reduce(out=sq_t[:], in_=sqp[:],
                                            op=ALU.add, axis=AX.X)
                    mean = stp.tile([128, 1], f32, tag="mean")
                    nc.vector.tensor_scalar(out=mean[:], in0=s_t[:],
                                            scalar1=1.0 / SP, scalar2=None,
                                            op0=ALU.mult)
                    msq = stp.tile([128, 1], f32, tag="msq")
                    nc.vector.scalar_tensor_tensor(
                        out=msq[:], in0=mean[:], scalar=1.0, in1=s_t[:],
                        op0=ALU.mult, op1=ALU.mult)
                    var_ = stp.tile([128, 1], f32, tag="var")
                    nc.vector.scalar_tensor_tensor(
                        out=var_[:], in0=sq_t[:], scalar=1.0, in1=msq[:],
                        op0=ALU.mult, op1=ALU.subtract)
                    std = stp.tile([128, 1], f32, tag="std")
                    nc.scalar.activation(std[:], var_[:], AF.Sqrt,
                                         bias=eps_sb[:, 0:1],
                                         scale=1.0 / (SP - 1))
                    rstd = stp.tile([128, 1], f32, tag="rstd")
                    nc.vector.reciprocal(rstd[:], std[:])
                    nshift = stp.tile([128, 1], f32, tag="nshift")
                    nc.vector.scalar_tensor_tensor(
                        out=nshift[:], in0=mean[:], scalar=-1.0, in1=rstd[:],
                        op0=ALU.mult, op1=ALU.mult)
                    # normalize: ctile -> pad interior (DVE, strided write)
                    nc.vector.tensor_scalar(
                        out=interior,
                        in0=ctile[:].rearrange("p (a b) -> p a b", a=HW),
                        scalar1=rstd[:, 0:1], scalar2=nshift[:, 0:1],
                        op0=ALU.mult, op1=ALU.add)
                    # reflect borders
                    nc.vector.tensor_copy(pad[:, 0, 1:65], pad[:, 2, 1:65])
                    nc.vector.tensor_copy(pad[:, 65, 1:65], pad[:, 63, 1:65])
                    nc.vector.tensor_copy(pad[:, :, 0], pad[:, :, 2])
                    nc.vector.tensor_copy(pad[:, :, 65], pad[:, :, 63])

                    wT = lhsT[:, ts, n].rearrange("p a b c -> p a (b c)")
                    pwT = pwlT[:, ts, n].rearrange("p a b -> p (a b)")
                    for c in range(NCHUNK):
                        ps4 = psd.tile([128, 8, 64], f32, tag="psD")
                        for k in range(9):
                            dy, dx = divmod(k, 3)
                            nc.tensor.matmul(
                                ps4[:], wT[:, k, :],
                                pad[:, c * 8 + dy:c * 8 + dy + 8,
                                    dx:dx + 64],
                                start=(k == 0), stop=(k == 8))
                        dsb = wkp.tile([128, CK], bf, tag="dsb")
                        nc.vector.tensor_copy(
                            dsb[:].rearrange("p (a b) -> p a b", a=8), ps4[:])
                        ps5 = pse.tile([128, CK], f32, tag="psE")
                        nc.tensor.matmul(ps5[:], pwT, dsb[:],
                                         start=True, stop=True)
                        a_t = wkp.tile([128, CK], f32, tag="at")
                        b_t = wkp.tile([128, CK], f32, tag="bt")
                        ot = wkp.tile([128, CK], f32, tag="ot")
                        nc.scalar.activation(a_t[:], ps5[:], AF.Relu,
                                             bias=bias_sb[:, ts, n:n + 1])
                        nc.scalar.activation(b_t[:], ps5[:], AF.Relu,
                                             bias=negb_sb[:, ts, n:n + 1],
                                             scale=-1.0)
                        nc.vector.scalar_tensor_tensor(
                            out=ot[:], in0=b_t[:], scalar=-0.01, op0=ALU.mult,
                            in1=a_t[:], op1=ALU.add)
                        nc.gpsimd.tensor_tensor(
                            out=ot[:].rearrange("p (a b) -> p a b", a=8),
                            in0=ot[:].rearrange("p (a b) -> p a b", a=8),
                            in1=pad[:, 1 + c * 8:1 + c * 8 + 8, 1:65],
                            op=ALU.add)
                        nc.sync.dma_start(out=out_d[n, ts, :, c * CK:(c + 1) * CK],
                                          in_=ot[:])
    nc.compile()
    return nc


_NC_CACHE = None


def kernel(**inputs):
    global _NC_CACHE
    in_maps = _host_prep(**inputs)
    if _NC_CACHE is None:
        _NC_CACHE = _build_nc()
    nc = _NC_CACHE
    from concourse.bass_utils import run_bass_kernel_spmd
    res = run_bass_kernel_spmd(nc, in_maps, core_ids=list(range(N_CORES)))
    outs = []
    for c in range(N_CORES):
        o = res.results[c]["out"].reshape(NPC, TS, 128, SP)
        outs.append(o.reshape(NPC, CH, HW, HW))
    return np.concatenate(outs, axis=0).astype(np.float32)


# revision 18
# speedup vs baseline: 1.5238x; 1.1745x over previous
"""AdaConv (per-sample dynamic grouped conv) on 8 TRN2 NeuronCores.

Data parallel: batch 16 -> 8 cores x 2 samples. Per core:
  - predictor convs (tiny matmuls, bf16) produce per-sample dw (512,8,3,3)
    and pw (512,8) kernels + bias (512,), laid out directly as block-diagonal
    128x128 lhsT tiles via small scatter DMAs
  - instance norm (DVE/ACT) + reflect pad into a [128, 66, 66] SBUF tile
  - 3x3 grouped conv = 9 shifted-AP matmuls (f32r) accumulated in PSUM,
    pointwise grouped conv = 1 matmul, Lrelu(+bias) on ACT, +cn on DVE
Channel tiling: 512 ch = 4 tiles x 128 partitions; group g (8ch) is tile-local.
"""

import numpy as np

N_FULL = 16
N_CORES = 8
NPC = 2            # samples per core
TS = 4             # channel tiles of 128
CH = 512
HW = 64
SP = HW * HW       # 4096
PADW = HW + 2      # 66
EPS = 1e-5
NCHUNK = 8         # spatial chunks of 512 px (8 rows)
CK = SP // NCHUNK  # 512


def _host_prep(style_encoding, content_in, dw_w, dw_b, pw_kn_w, pw_kn_b,
               pw_bias_w, pw_bias_b):
    """Layout-only transforms. Returns list of 8 per-core input dicts."""
    import ml_dtypes
    bf16 = ml_dtypes.bfloat16

    ts_i = np.arange(TS)[:, None, None]
    o_i = np.arange(8)[None, :, None]
    m_i = np.arange(128)[None, None, :]
    OC = 8 * (128 * ts_i + 8 * (m_i // 8) + o_i) + (m_i % 8)  # (4,8,128)

    # dw predictor weights -> block-diag-of-two-64 lhsT tiles (bf16)
    W = dw_w[OC]                                  # (4,8,128m,64ic,2,2)
    Wk = np.transpose(W, (0, 1, 4, 5, 3, 2)).reshape(TS, 8, 4, 64, 128)
    dwp = np.zeros((TS, 8, 4, 128, 128), np.float32)
    dwp[:, :, :, 0:64, 0:64] = Wk[:, :, :, :, 0:64]
    dwp[:, :, :, 64:128, 64:128] = Wk[:, :, :, :, 64:128]
    dwp = dwp.reshape(32, 4, 128, 128).astype(bf16)

    # pw predictor weights (scaled by 1/16 to fold the style spatial mean)
    Wp = np.transpose(pw_kn_w[OC, :, 0, 0] / 16.0, (0, 1, 3, 2))  # (4,8,64,128)
    pwp = np.zeros((TS, 8, 128, 128), np.float32)
    pwp[:, :, 0:64, 0:64] = Wp[:, :, :, 0:64]
    pwp[:, :, 64:128, 64:128] = Wp[:, :, :, 64:128]
    pwp = pwp.reshape(32, 128, 128).astype(bf16)

    pwbT = np.ascontiguousarray(
        (pw_bias_w[:, :, 0, 0].T / 16.0).reshape(TS, 128, CH)
        .transpose(1, 0, 2)).astype(bf16)                        # (128,4,512)

    p_i = np.arange(128)[:, None, None]
    OC2 = 8 * (128 * np.arange(TS)[None, :, None] + 8 * (p_i // 8)
               + np.arange(8)[None, None, :]) + (p_i % 8)        # (128,4,8)
    dwb = dw_b[OC2].reshape(128, 32).astype(np.float32)
    pwkb = pw_kn_b[OC2].reshape(128, 32).astype(np.float32)
    pbb = np.ascontiguousarray(pw_bias_b.reshape(TS, 128).T)     # (128,4)

    in_maps = []
    for c in range(N_CORES):
        sl = slice(c * NPC, (c + 1) * NPC)
        style_core = np.asarray(style_encoding[sl])              # (2,512,4,4)
        sd = style_core.reshape(NPC, TS, 128, 4, 4).transpose(2, 1, 0, 3, 4)
        content_core = np.ascontiguousarray(
            np.asarray(content_in[sl]).reshape(NPC, TS, 128, SP))
        in_maps.append({
            "style": np.ascontiguousarray(sd).astype(bf16),
            "content": content_core.astype(np.float32),
            "dwp": dwp, "pwp": pwp, "pwbT": pwbT,
            "dwb": dwb, "pwkb": pwkb, "pbb": pbb,
            "zeros": np.zeros((128, 9216), np.float32),
        })
    return in_maps


def _emulate_core(m):
    """Numpy mimic of the device graph (f32) for one core's in_map."""
    style = np.asarray(m["style"], np.float32).transpose(1, 0, 2, 3, 4)  # (4,128,2,4,4)
    content = m["content"]                         # (2,4,128,4096)
    dwp = np.asarray(m["dwp"], np.float32).reshape(TS, 8, 4, 128, 128)
    pwp = np.asarray(m["pwp"], np.float32).reshape(TS, 8, 128, 128)
    pwbT = np.asarray(m["pwbT"], np.float32)
    dwb, pwkb, pbb = m["dwb"], m["pwkb"], m["pbb"]

    sd = style.sum(axis=(3, 4))                    # (4,128,2)  (sum, not mean)

    lhsT = np.zeros((128, TS, NPC, 9, 16, 8), np.float32)
    pwlT = np.zeros((128, TS, NPC, 16, 8), np.float32)
    bias_sb = np.zeros((128, TS, NPC), np.float32)

    for ts in range(TS):
        for o in range(8):
            mt = ts * 8 + o
            psA = np.zeros((128, NPC, 3, 3), np.float32)
            for koff in range(4):
                ky, kx = divmod(koff, 2)
                rhs = style[ts, :, :, ky:ky + 3, kx:kx + 3]     # (128,2,3,3)
                psA += np.einsum('rm,rnab->mnab', dwp[ts, o, koff], rhs)
            S = np.maximum(psA + dwb[:, mt][:, None, None, None], 0.0)
            psB = np.einsum('rm,rn->mn', pwp[ts, o], sd[ts])    # (128,2)
            Spw = np.maximum(psB + pwkb[:, mt][:, None], 0.0)
            for G in range(16):
                r = slice(8 * G, 8 * G + 8)
                # S[p,n,dy,dx] -> lhsT[p, ts, n, k, G, o]
                lhsT[r, ts, :, :, G, o] = S[r].reshape(8, NPC, 9)
                pwlT[r, ts, :, G, o] = Spw[r]
    for ts in range(TS):
        psC = np.zeros((128, NPC), np.float32)
        for kt in range(TS):
            psC += np.einsum('rm,rn->mn',
                             pwbT[:, kt, 128 * ts:128 * (ts + 1)], sd[kt])
        bias_sb[:, ts, :] = np.maximum(psC + pbb[:, ts][:, None], 0.0)

    out = np.zeros((NPC, TS, 128, SP), np.float32)
    for n in range(NPC):
        for ts in range(TS):
            x = content[n, ts]                                  # (128,4096)
            s = x.sum(axis=1)
            sq = (x * x).sum(axis=1)
            mean = s / SP
            var = (sq - mean * s) / (SP - 1)
            rstd = 1.0 / np.sqrt(var + EPS)
            cn = (x - mean[:, None]) * rstd[:, None]
            cn2 = cn.reshape(128, HW, HW)
            pad = np.zeros((128, PADW, PADW), np.float32)
            pad[:, 1:65, 1:65] = cn2
            pad[:, 0, 1:65] = pad[:, 2, 1:65]
            pad[:, 65, 1:65] = pad[:, 63, 1:65]
            pad[:, :, 0] = pad[:, :, 2]
            pad[:, :, 65] = pad[:, :, 63]
            for c in range(NCHUNK):
                psD = np.zeros((128, 8, 64), np.float32)
                for k in range(9):
                    dy, dx = divmod(k, 3)
                    W128 = np.zeros((128, 128), np.float32)
                    W128 = lhsT[:, ts, n, k, :, :].reshape(128, 128)
                    rhs = pad[:, c * 8 + dy:c * 8 + dy + 8, dx:dx + 64]
                    psD += np.einsum('rm,rab->mab', W128, rhs)
                d = psD.reshape(128, CK)
                psE = np.einsum('rm,rq->mq', pwlT[:, ts, n].reshape(128, 128), d)
                t = psE + bias_sb[:, ts, n][:, None]
                t = np.where(t >= 0, t, 0.01 * t)
                out[n, ts, :, c * CK:(c + 1) * CK] = \
                    t + cn[:, c * CK:(c + 1) * CK]
    return out


def _build_nc():
    import concourse.bass as bass
    import concourse.mybir as mybir
    from concourse import bacc
    from concourse.tile import TileContext

    f32, bf, f32r = mybir.dt.float32, mybir.dt.bfloat16, mybir.dt.float32r
    AF = mybir.ActivationFunctionType
    ALU = mybir.AluOpType
    AX = mybir.AxisListType

    nc = bacc.Bacc()
    style_d = nc.declare_dram_parameter("style", [128, TS, NPC, 4, 4], bf, False)
    content_d = nc.declare_dram_parameter("content", [NPC, TS, 128, SP], f32r, False)
    dwp_d = nc.declare_dram_parameter("dwp", [32, 4, 128, 128], bf, False)
    pwp_d = nc.declare_dram_parameter("pwp", [32, 128, 128], bf, False)
    pwbT_d = nc.declare_dram_parameter("pwbT", [128, TS, CH], bf, False)
    dwb_d = nc.declare_dram_parameter("dwb", [128, 32], f32, False)
    pwkb_d = nc.declare_dram_parameter("pwkb", [128, 32], f32, False)
    pbb_d = nc.declare_dram_parameter("pbb", [128, TS], f32, False)
    zeros_d = nc.declare_dram_parameter("zeros", [128, 9216], f32r, False)
    out_d = nc.declare_dram_parameter("out", [NPC, TS, 128, SP], f32, True)

    with TileContext(nc) as tc:
        with (
            tc.tile_pool(name="persist", bufs=1) as pp,
            tc.tile_pool(name="wstream", bufs=3) as wp,
            tc.tile_pool(name="pads", bufs=3) as padp,
            tc.tile_pool(name="work", bufs=4) as wkp,
            tc.tile_pool(name="stats", bufs=4) as stp,
            tc.tile_pool(name="psA", bufs=2, space="PSUM") as psa,
            tc.tile_pool(name="psD", bufs=2, space="PSUM") as psd,
            tc.tile_pool(name="psE", bufs=2, space="PSUM") as pse,
        ):
            style_sb = pp.tile([128, TS, NPC, 4, 4], bf, tag="style")
            dwb_sb = pp.tile([128, 32], f32, tag="dwb")
            pwkb_sb = pp.tile([128, 32], f32, tag="pwkb")
            pbb_sb = pp.tile([128, TS], f32, tag="pbb")
            pwbT_sb = pp.tile([128, TS, CH], bf, tag="pwbT")
            sd_f = pp.tile([128, TS, NPC], f32, tag="sdf")
            sd_sb = pp.tile([128, TS, NPC], bf, tag="sd")
            lhsT = pp.tile([128, TS, NPC, 9, 16, 8], f32r, tag="lhsT")
            pwlT = pp.tile([128, TS, NPC, 16, 8], bf, tag="pwlT")
            S_dw = pp.tile([128, TS, NPC, 9, 8], f32r, tag="Sdw")
            S_pw = pp.tile([128, TS, NPC, 8], bf, tag="Spw")
            bias_sb = pp.tile([128, TS, NPC], f32, tag="bias")
            negb_sb = pp.tile([128, TS, NPC], f32, tag="negb")
            eps_sb = pp.tile([128, 1], f32, tag="eps")
            nc.vector.memset(eps_sb[:], EPS)

            nc.sync.dma_start(
                out=style_sb[:].rearrange("p a n y x -> p (a n y x)"),
                in_=style_d[:].rearrange("p a n y x -> p (a n y x)"))
            nc.sync.dma_start(out=dwb_sb[:], in_=dwb_d[:])
            nc.sync.dma_start(out=pwkb_sb[:], in_=pwkb_d[:])
            nc.sync.dma_start(out=pbb_sb[:], in_=pbb_d[:])
            nc.sync.dma_start(
                out=pwbT_sb[:].rearrange("p a b -> p (a b)"),
                in_=pwbT_d[:].rearrange("p a b -> p (a b)"))

            nc.vector.memset(pwlT[:].rearrange("p a n g o -> p (a n g o)"), 0.0)

            # style spatial sum -> sd (x 1/16 folded into pw weights)
            for ts in range(TS):
                nc.vector.tensor_reduce(
                    out=sd_f[:, ts, :], in_=style_sb[:, ts, :, :, :],
                    op=ALU.add, axis=AX.XY)
            nc.vector.tensor_copy(
                sd_sb[:].rearrange("p a n -> p (a n)"),
                sd_f[:].rearrange("p a n -> p (a n)"))

            # ---- predictor matmuls ----
            for ts in range(TS):
                for o in range(8):
                    mt = ts * 8 + o
                    dmat = wp.tile([128, 4, 128], bf, tag="dmat")
                    nc.sync.dma_start(out=dmat[:],
                                      in_=dwp_d[mt].transpose([1, 0, 2]))
                    ps = psa.tile([128, NPC, 3, 3], f32, tag="psA")
                    for koff in range(4):
                        ky, kx = divmod(koff, 2)
                        nc.tensor.matmul(
                            ps[:], dmat[:, koff, :],
                            style_sb[:, ts, :, ky:ky + 3, kx:kx + 3],
                            start=(koff == 0), stop=(koff == 3))
                    nc.scalar.activation(
                        S_dw[:, ts, :, :, o], ps[:], AF.Relu,
                        bias=dwb_sb[:, mt:mt + 1])

                    pmat = wp.tile([128, 128], bf, tag="pmat")
                    nc.sync.dma_start(out=pmat[:], in_=pwp_d[mt])
                    ps2 = psa.tile([128, NPC], f32, tag="psA")
                    nc.tensor.matmul(ps2[:], pmat[:], sd_sb[:, ts, :],
                                     start=True, stop=True)
                    nc.scalar.activation(
                        S_pw[:, ts, :, o], ps2[:], AF.Relu,
                        bias=pwkb_sb[:, mt:mt + 1])

            for ts in range(TS):
                ps3 = psa.tile([128, NPC], f32, tag="psA")
                for kt in range(TS):
                    nc.tensor.matmul(
                        ps3[:], pwbT_sb[:, kt, 128 * ts:128 * (ts + 1)],
                        sd_sb[:, kt, :], start=(kt == 0), stop=(kt == 3))
                nc.scalar.activation(bias_sb[:, ts, :], ps3[:], AF.Relu,
                                     bias=pbb_sb[:, ts:ts + 1])
                nc.vector.tensor_scalar(out=negb_sb[:, ts, :],
                                        in0=bias_sb[:, ts, :], scalar1=-1.0,
                                        scalar2=None, op0=ALU.mult)

            nc.sync.dma_start(
                out=lhsT[:].rearrange("p a n k g o -> p (a n k g o)"),
                in_=zeros_d[:, 0:9216])

            # ---- scatter into block-diagonal lhsT tiles ----
            for ts in range(TS):
                for G in range(16):
                    r = slice(8 * G, 8 * G + 8)
                    nc.sync.dma_start(out=lhsT[r, ts, :, :, G, :],
                                      in_=S_dw[r, ts, :, :, :])
                    nc.sync.dma_start(out=pwlT[r, ts, :, G, :],
                                      in_=S_pw[r, ts, :, :])

            # ---- main per (n, ts): prologue pipelined one tile ahead ----
            def prologue(n, ts):
                pad = padp.tile([128, PADW, PADW], f32r, tag="pad")
                interior = pad[:, 1:65, 1:65]
                ctile = padp.tile([128, SP], f32r, tag="ctile")
                nc.sync.dma_start(out=ctile[:], in_=content_d[n, ts])

                s_t = stp.tile([128, 1], f32, tag="sum")
                sq_t = stp.tile([128, 1], f32, tag="sumsq")
                nc.vector.tensor_reduce(out=s_t[:], in_=ctile[:],
                                        op=ALU.add, axis=AX.X)
                sqscr = wkp.tile([128, CK], f32, tag="sqscr")
                sqp = stp.tile([128, NCHUNK], f32, tag="sqp")
                for c in range(NCHUNK):
                    nc.scalar.activation(
                        sqscr[:], ctile[:, c * CK:(c + 1) * CK],
                        AF.Square, accum_out=sqp[:, c:c + 1])
                nc.vector.tensor_reduce(out=sq_t[:], in_=sqp[:],
                                        op=ALU.add, axis=AX.X)
                mean = stp.tile([128, 1], f32, tag="mean")
                nc.vector.tensor_scalar(out=mean[:], in0=s_t[:],
                                        scalar1=1.0 / SP, scalar2=None,
                                        op0=ALU.mult)
                msq = stp.tile([128, 1], f32, tag="msq")
                nc.vector.scalar_tensor_tensor(
                    out=msq[:], in0=mean[:], scalar=1.0, in1=s_t[:],
                    op0=ALU.mult, op1=ALU.mult)
                var_ = stp.tile([128, 1], f32, tag="var")
                nc.vector.scalar_tensor_tensor(
                    out=var_[:], in0=sq_t[:], scalar=1.0, in1=msq[:],
                    op0=ALU.mult, op1=ALU.subtract)
                std = stp.tile([128, 1], f32, tag="std")
                nc.scalar.activation(std[:], var_[:], AF.Sqrt,
                                     bias=eps_sb[:, 0:1],
                                     scale=1.0 / (SP - 1))
                rstd = stp.tile([128, 1], f32, tag="rstd")
                nc.vector.reciprocal(rstd[:], std[:])
                nshift = stp.tile([128, 1], f32, tag="nshift")
                nc.vector.scalar_tensor_tensor(
                    out=nshift[:], in0=mean[:], scalar=-1.0, in1=rstd[:],
                    op0=ALU.mult, op1=ALU.mult)
                nc.vector.tensor_scalar(
                    out=interior,
                    in0=ctile[:].rearrange("p (a b) -> p a b", a=HW),
                    scalar1=rstd[:, 0:1], scalar2=nshift[:, 0:1],
                    op0=ALU.mult, op1=ALU.add)
                nc.vector.tensor_copy(pad[:, 0, 1:65], pad[:, 2, 1:65])
                nc.vector.tensor_copy(pad[:, 65, 1:65], pad[:, 63, 1:65])
                nc.vector.tensor_copy(pad[:, :, 0], pad[:, :, 2])
                nc.vector.tensor_copy(pad[:, :, 65], pad[:, :, 63])
                return pad

            def conv(n, ts, pad):
                wT = lhsT[:, ts, n].rearrange("p a b c -> p a (b c)")
                pwT = pwlT[:, ts, n].rearrange("p a b -> p (a b)")
                for c in range(NCHUNK):
                    ps4 = psd.tile([128, 8, 64], f32, tag="psD")
                    for k in range(9):
                        dy, dx = divmod(k, 3)
                        nc.tensor.matmul(
                            ps4[:], wT[:, k, :],
                            pad[:, c * 8 + dy:c * 8 + dy + 8,
                                dx:dx + 64],
                            start=(k == 0), stop=(k == 8))
                    dsb = wkp.tile([128, CK], bf, tag="dsb")
                    nc.vector.tensor_copy(
                        dsb[:].rearrange("p (a b) -> p a b", a=8), ps4[:])
                    ps5 = pse.tile([128, CK], f32, tag="psE")
                    nc.tensor.matmul(ps5[:], pwT, dsb[:],
                                     start=True, stop=True)
                    a_t = wkp.tile([128, CK], f32, tag="at")
                    b_t = wkp.tile([128, CK], f32, tag="bt")
                    ot = wkp.tile([128, CK], f32, tag="ot")
                    nc.scalar.activation(a_t[:], ps5[:], AF.Relu,
                                         bias=bias_sb[:, ts, n:n + 1])
                    nc.scalar.activation(b_t[:], ps5[:], AF.Relu,
                                         bias=negb_sb[:, ts, n:n + 1],
                                         scale=-1.0)
                    nc.vector.scalar_tensor_tensor(
                        out=ot[:], in0=b_t[:], scalar=-0.01, op0=ALU.mult,
                        in1=a_t[:], op1=ALU.add)
                    nc.gpsimd.tensor_tensor(
                        out=ot[:].rearrange("p (a b) -> p a b", a=8),
                        in0=ot[:].rearrange("p (a b) -> p a b", a=8),
                        in1=pad[:, 1 + c * 8:1 + c * 8 + 8, 1:65],
                        op=ALU.add)
                    nc.sync.dma_start(
                        out=out_d[n, ts, :, c * CK:(c + 1) * CK], in_=ot[:])

            order = [(n, ts) for n in range(NPC) for ts in range(TS)]
            pending = {order[0]: prologue(*order[0])}
            for i, key in enumerate(order):
                if i + 1 < len(order):
                    pending[order[i + 1]] = prologue(*order[i + 1])
                conv(key[0], key[1], pending.pop(key))
    nc.compile()
    return nc


_NC_CACHE = None


def kernel(**inputs):
    global _NC_CACHE
    in_maps = _host_prep(**inputs)
    if _NC_CACHE is None:
        _NC_CACHE = _build_nc()
    nc = _NC_CACHE
    from concourse.bass_utils import run_bass_kernel_spmd
    res = run_bass_kernel_spmd(nc, in_maps, core_ids=list(range(N_CORES)))
    outs = []
    for c in range(N_CORES):
        o = res.results[c]["out"].reshape(NPC, TS, 128, SP)
        outs.append(o.reshape(NPC, CH, HW, HW))
    return np.concatenate(outs, axis=0).astype(np.float32)
